# revision 2
# baseline (speedup 1.0000x reference)
"""Trainium2 Bass kernel for BipolarSAE (top-k masking sparse autoencoder).

reference:
    pre = x @ W_enc.T + b_enc          # [N, 4096]
    keep top-32 of |pre| per row (mask), f = pre * mask
    recon = f @ W_dec.T + b_dec        # [N, 768]
    returns (recon, f)

Strategy (8 NeuronCores, data-parallel over the 32768 tokens, 4096 each):
  Phase 1 (W_enc resident fp32): per 128-token block, encoder matmul in
    native fp32 (selection-critical precision), ACT evacuates PSUM as both
    pre and pre^2; VectorE extracts the 32nd-largest square via 4x max8 +
    3x match_replace, then one scalar_tensor_tensor applies the threshold
    mask: f = (sq >= tau^2) * pre.  f goes to DRAM.
  Phase 2 (W_dec resident float32r): f read back per block, PE-transposed
    to features-on-partitions, decoder matmul in float32r (4x faster than
    fp32; decoder precision is not selection-critical).
Biases are folded in as K=1 matmuls with a ones row-vector.
"""

import os
import sys

sys.path.insert(0, "/opt/trn_rl_repo")

import numpy as np

import concourse.bacc as bacc
import concourse.bass as bass
import concourse.mybir as mybir
import concourse.tile as tile
from concourse.bass import ts
from concourse.masks import make_identity

P = 128
D_IN = 768
D_OUT = 4096
K_TOP = 32
N_TOKENS = 32768
N_CORES = 8
T_CORE = N_TOKENS // N_CORES  # 4096 tokens per core

KO = D_IN // P  # 6 contraction chunks (encoder)
NSL = 8  # encoder feature slices
SL = D_OUT // NSL  # 512
FC = D_OUT // P  # 32 feature chunks (decoder contraction)

LAST_RESULTS = None  # test harness reads exec_time_ns from here

f32 = mybir.dt.float32
f32r = mybir.dt.float32r


def build(t_core: int) -> bacc.Bacc:
    nblk = t_core // P
    nc = bacc.Bacc("TRN2", target_bir_lowering=False, debug=False)

    xT = nc.declare_dram_parameter("xT", [D_IN, t_core], f32, isOutput=False)
    wencT = nc.declare_dram_parameter("wencT", [D_IN, D_OUT], f32, isOutput=False)
    benc = nc.declare_dram_parameter("benc", [1, D_OUT], f32, isOutput=False)
    wdecT = nc.declare_dram_parameter("wdecT", [D_OUT, D_IN], f32r, isOutput=False)
    bdec = nc.declare_dram_parameter("bdec", [1, D_IN], f32r, isOutput=False)

    f_out = nc.declare_dram_parameter("f", [t_core, D_OUT], f32, isOutput=True)
    recon_out = nc.declare_dram_parameter("recon", [t_core, D_IN], f32, isOutput=True)

    xT_t = xT.ap().rearrange("(o p) t -> p o t", p=P)  # [128, 6, t_core]

    with tile.TileContext(nc) as tc:
        # ---------------- Phase 1: encode + top-k mask ----------------
        with (
            tc.tile_pool(name="p1w", bufs=1) as wpool,
            tc.tile_pool(name="p1xt", bufs=2) as xtpool,
            tc.tile_pool(name="p1dbl", bufs=2) as dpool,
            tc.tile_pool(name="p1sgl", bufs=1) as spool,
            tc.tile_pool(name="p1ps", bufs=8, space="PSUM") as ppool,
        ):
            wenc_sb = wpool.tile([P, KO, D_OUT], f32)
            nc.sync.dma_start(wenc_sb[:], wencT.ap().rearrange("(o p) f -> p o f", p=P))
            benc_sb = wpool.tile([1, D_OUT], f32)
            nc.sync.dma_start(benc_sb[:], benc.ap())
            ones_sb = wpool.tile([1, P], f32)
            nc.vector.memset(ones_sb[:], 1.0)

            for b in range(nblk):
                xt = xtpool.tile([P, KO, P], f32, name=f"xt{b}", tag="xt")
                nc.sync.dma_start(xt[:], xT_t[:, :, ts(b, P)])

                pre = dpool.tile([P, D_OUT], f32, name=f"pre{b}", tag="pre")
                sq = spool.tile([P, D_OUT], f32, name=f"sq{b}", tag="sq")
                for s in range(NSL):
                    ps = ppool.tile([P, SL], f32, name=f"eps{b}_{s}", tag="eps")
                    for ko in range(KO):
                        nc.tensor.matmul(
                            ps[:],
                            xt[:, ko, :],
                            wenc_sb[:, ko, ts(s, SL)],
                            start=(ko == 0),
                            stop=False,
                        )
                    nc.tensor.matmul(
                        ps[:], ones_sb[:], benc_sb[:, ts(s, SL)], start=False, stop=True
                    )
                    nc.scalar.activation(
                        sq[:, ts(s, SL)], ps[:], mybir.ActivationFunctionType.Square
                    )
                    nc.scalar.copy(pre[:, ts(s, SL)], ps[:])

                # top-32 threshold (on squares): 4x max8 + 3x match_replace
                zap = spool.tile([P, D_OUT], f32, name=f"zap{b}", tag="zap")
                m8 = dpool.tile([P, 4, 8], f32, name=f"m8{b}", tag="m8")
                nc.vector.max(out=m8[:, 0, :], in_=sq[:])
                nc.vector.match_replace(
                    out=zap[:], in_to_replace=m8[:, 0, :], in_values=sq[:], imm_value=-1.0
                )
                for r in range(1, 4):
                    nc.vector.max(out=m8[:, r, :], in_=zap[:])
                    if r < 3:
                        nc.vector.match_replace(
                            out=zap[:],
                            in_to_replace=m8[:, r, :],
                            in_values=zap[:],
                            imm_value=-1.0,
                        )

                f_sb = spool.tile([P, D_OUT], f32, name=f"f{b}", tag="fsb")
                nc.vector.scalar_tensor_tensor(
                    out=f_sb[:],
                    in0=sq[:],
                    scalar=m8[:, 3, 7:8],
                    in1=pre[:],
                    op0=mybir.AluOpType.is_ge,
                    op1=mybir.AluOpType.mult,
                )
                nc.sync.dma_start(f_out.ap()[ts(b, P), :], f_sb[:])

        tc.strict_bb_all_engine_barrier()

        # ---------------- Phase 2: decode ----------------
        with (
            tc.tile_pool(name="p2w", bufs=1) as wpool2,
            tc.tile_pool(name="p2fin", bufs=2) as finpool,
            tc.tile_pool(name="p2ft", bufs=2) as ftpool,
            tc.tile_pool(name="p2rec", bufs=2) as recpool,
            tc.tile_pool(name="p2ps", bufs=2, space="PSUM") as rpspool,
            tc.tile_pool(name="p2pst", bufs=4, space="PSUM") as tpspool,
        ):
            wdec_sb = wpool2.tile([P, FC, D_IN], f32r)
            nc.sync.dma_start(wdec_sb[:], wdecT.ap().rearrange("(o p) d -> p o d", p=P))
            bdec_sb = wpool2.tile([1, D_IN], f32r)
            nc.sync.dma_start(bdec_sb[:], bdec.ap())
            ones2 = wpool2.tile([1, P], f32)
            nc.vector.memset(ones2[:], 1.0)
            ones2_r = wpool2.tile([1, P], f32r)
            nc.vector.tensor_copy(ones2_r[:], ones2[:])
            ident = wpool2.tile([P, P], f32)
            make_identity(nc, ident[:])

            for b in range(nblk):
                fin = finpool.tile([P, D_OUT], f32, name=f"fin{b}", tag="fin")
                nc.sync.dma_start(fin[:], f_out.ap()[ts(b, P), :])

                fT = ftpool.tile([P, FC, P], f32r, name=f"fT{b}", tag="fT")
                for c in range(FC):
                    pst = tpspool.tile([P, P], f32, name=f"pst{b}_{c}", tag="pst")
                    nc.tensor.transpose(pst[:], fin[:, ts(c, P)], ident[:])
                    nc.scalar.copy(fT[:, c, :], pst[:])

                rps = rpspool.tile([P, D_IN], f32, name=f"rps{b}", tag="rps")
                for n0, n1 in ((0, 512), (512, 768)):
                    for c in range(FC):
                        nc.tensor.matmul(
                            rps[:, n0:n1],
                            fT[:, c, :],
                            wdec_sb[:, c, n0:n1],
                            start=(c == 0),
                            stop=False,
                        )
                    nc.tensor.matmul(
                        rps[:, n0:n1],
                        ones2_r[:],
                        bdec_sb[:, n0:n1],
                        start=False,
                        stop=True,
                    )
                rec = recpool.tile([P, D_IN], f32, name=f"rec{b}", tag="rec")
                nc.scalar.copy(rec[:], rps[:])
                nc.sync.dma_start(recon_out.ap()[ts(b, P), :], rec[:])

    nc.compile()
    return nc


_BUILT = {}


def _get_built(t_core: int):
    if t_core not in _BUILT:
        _BUILT[t_core] = build(t_core)
    return _BUILT[t_core]


def _install_ntff_shim():
    """The image's antenv lacks axon_hooks; synthesize it from trn_agent_boot
    so run_bass_kernel_spmd(trace=True) can capture NTFF profiles."""
    import types

    if "antenv.axon_hooks" in sys.modules:
        return
    try:
        from trn_agent_boot.trn_boot import _ntff_profile_via_ctypes

        hook = _ntff_profile_via_ctypes("/opt/axon/libaxon_pjrt.so")
        mod = types.ModuleType("antenv.axon_hooks")
        mod.get_axon_ntff_profile_hook = lambda: hook
        sys.modules["antenv.axon_hooks"] = mod
    except Exception:
        pass


def kernel(x, W_enc, b_enc, W_dec, b_dec):
    global LAST_RESULTS
    from concourse.bass_utils import run_bass_kernel_spmd

    if os.environ.get("SAE_TRACE"):
        _install_ntff_shim()

    x = np.asarray(x, dtype=np.float32)
    W_enc = np.asarray(W_enc, dtype=np.float32)
    b_enc = np.asarray(b_enc, dtype=np.float32)
    W_dec = np.asarray(W_dec, dtype=np.float32)
    b_dec = np.asarray(b_dec, dtype=np.float32)

    n_tokens = x.shape[0]
    t_core = n_tokens // N_CORES
    nc = _get_built(t_core)

    xT = np.ascontiguousarray(x.T)  # [768, N]
    wencT = np.ascontiguousarray(W_enc.T)  # [768, 4096]
    wdecT = np.ascontiguousarray(W_dec.T)  # [4096, 768]
    benc2 = np.ascontiguousarray(b_enc[None, :])
    bdec2 = np.ascontiguousarray(b_dec[None, :])

    in_maps = [
        {
            "xT": np.ascontiguousarray(xT[:, i * t_core : (i + 1) * t_core]),
            "wencT": wencT,
            "benc": benc2,
            "wdecT": wdecT,
            "bdec": bdec2,
        }
        for i in range(N_CORES)
    ]

    res = run_bass_kernel_spmd(
        nc,
        in_maps,
        list(range(N_CORES)),
        trace=bool(os.environ.get("SAE_TRACE")),
    )
    LAST_RESULTS = res

    recon = np.concatenate([res.results[i]["recon"] for i in range(N_CORES)], axis=0)
    f = np.concatenate([res.results[i]["f"] for i in range(N_CORES)], axis=0)
    return recon, f
